# revision 18
# baseline (speedup 1.0000x reference)
"""AlignmentModule kernel for 8 TRN2 NeuronCores.

Sharding: data-parallel over batch B=8 -> one batch element per core.

Per-core math (validated numerically against the reference):
  spk_t = text_spk_w @ e ; spk_f = feat_spk_w @ e
  texts_c = texts.T + spk_t[:,None] ; feats_c = feats.T + spk_f[:,None]
  te = conv1x(relu(conv3(texts_c)))                       (256, 1024)
  fe = 2T * conv1x(relu(conv3(relu(conv3(feats_c)))))     (256, 4096)  [2T folded into w3/b3]
  t2[t] = sum_c te[c,t]^2
  s[f,t] = fe.T @ te - T*t2[t]          == -T*dist + T*f2  (f2 cancels in both softmaxes)
  logZ1 = ln(sum_t exp(s))              (s in [-0.25, 0] -> no max-shift needed)
  alp = s - logZ1 + lp                  -> output attn_logprob   [lp = ln(prior+eps), host-prepped]
  attn = (exp(alp) * mask01) / sum_t(exp(alp) * mask01)   -> output attn

The feat conv chain is software-pipelined with the attention chunks over
8 t-windows of 512 so PE/ACT/DVE/DMA overlap and the PE HAM clock-gate
stays warm (2.4 GHz).
"""

import sys

import numpy as np

if "/opt/trn_rl_repo" not in sys.path:
    sys.path.append("/opt/trn_rl_repo")

import ml_dtypes

import concourse.bass as bass
import concourse.bacc as bacc
import concourse.mybir as mybir
import concourse.tile as tile
from concourse import bass_utils
from concourse.alu_op_type import AluOpType

F32 = mybir.dt.float32
F16 = mybir.dt.float16
BF16 = mybir.dt.bfloat16
AF = mybir.ActivationFunctionType

B, T_TEXT, T_FEATS, ADIM, ODIM = 8, 1024, 4096, 256, 80
TEMPERATURE = 0.0005
EPS = 1e-8
NCORES = 8
FCH = T_FEATS // 128  # 32 attention row-chunks per core
NW = 512              # matmul moving width
NWIN = T_FEATS // NW  # 8 feat windows, 4 chunks each

# packed-weight column layout (all lhsT, cin rows x concatenated cols)
WPACK_OFFS = {}
_off = 0
for _nm, _w in [("tw1", 3 * ADIM), ("tw2", ADIM), ("fw2", 3 * ADIM), ("fw3", ADIM),
                ("tspk", ADIM), ("fspk", ODIM), ("spk", 1), ("fw1", 3 * ADIM)]:
    WPACK_OFFS[_nm] = _off
    _off += _w
WPACK_W = _off


def _patched_tables(arch):
    """Steer bacc's act-table pass to the one set containing every function
    we use (Exp, Ln, Identity, Relu, Copy) so the kernel does a single
    ACT_TABLE_LOAD instead of ping-ponging sets every chunk (~2.7us each).
    Set ids stay positional (full dict, original order) - only membership of
    the other sets is masked so the chooser can't pick them."""
    t = _orig_tables(arch)
    need = {AF.Exp, AF.Ln, AF.Identity, AF.Relu, AF.Copy}
    return {name: (set(fns) if name == "natural_log_exp_and_others"
                   else set(fns) - need)
            for name, fns in t.items()}


_orig_tables = bacc.get_activation_tables


def build_program():
    bacc.get_activation_tables = _patched_tables
    try:
        return _build_program_inner()
    finally:
        bacc.get_activation_tables = _orig_tables


def _build_program_inner():
    nc = bacc.Bacc("TRN2", target_bir_lowering=False, debug=False)

    # ---- DRAM I/O ----
    texts = nc.dram_tensor("texts", [T_TEXT, ADIM], BF16, kind="ExternalInput").ap()
    # feats padded 80 -> 128 channels on host (DMA transpose needs free%128==0)
    feats = nc.dram_tensor("feats", [T_FEATS, 128], BF16, kind="ExternalInput").ap()
    lp_in = nc.dram_tensor("lp", [T_FEATS, T_TEXT], F16, kind="ExternalInput").ap()
    pm_in = nc.dram_tensor("pm", [T_FEATS, T_TEXT], F16, kind="ExternalInput").ap()
    wpack = nc.dram_tensor("wpack", [ADIM, WPACK_W], BF16, kind="ExternalInput").ap()
    bpack = nc.dram_tensor("bpack", [ADIM, 5], F32, kind="ExternalInput").ap()

    attn_out = nc.dram_tensor("attn", [T_FEATS, T_TEXT], F16, kind="ExternalOutput").ap()
    alp_out = nc.dram_tensor("alp", [T_FEATS, T_TEXT], F16, kind="ExternalOutput").ap()

    with tile.TileContext(nc) as tc:
        with (
            tc.tile_pool(name="wpool", bufs=1) as wp,
            tc.tile_pool(name="actpool", bufs=1) as ap_,
            tc.tile_pool(name="chunk", bufs=4) as cp,
            tc.tile_pool(name="cols", bufs=6) as colp,
            tc.tile_pool(name="convps", bufs=2, space="PSUM") as convps,
            tc.tile_pool(name="spsum", bufs=3, space="PSUM") as spsum,
        ):
            # ---- load weights / constants into SBUF (packed: 4 DMAs) ----
            wpk = [wp.tile([128, WPACK_W], BF16, tag=f"wpk_{g}", name=f"wpk_{g}")
                   for g in range(2)]
            bpk = [wp.tile([128, 5], F32, tag=f"bpk_{g}", name=f"bpk_{g}")
                   for g in range(2)]
            for g in range(2):
                nc.sync.dma_start(wpk[g][:], wpack[g * 128:(g + 1) * 128, :])
                nc.sync.dma_start(bpk[g][:], bpack[g * 128:(g + 1) * 128, :])

            def wcut(width):
                off = [0]
                def cut(g):
                    o = off[0]
                    return None  # placeholder
                return None

            _o = WPACK_OFFS
            tw1_s = [wpk[g][:, _o["tw1"]:_o["tw1"] + 3 * ADIM] for g in range(2)]
            tw2_s = [wpk[g][:, _o["tw2"]:_o["tw2"] + ADIM] for g in range(2)]
            fw2_s = [wpk[g][:, _o["fw2"]:_o["fw2"] + 3 * ADIM] for g in range(2)]
            fw3_s = [wpk[g][:, _o["fw3"]:_o["fw3"] + ADIM] for g in range(2)]
            tspk_s = [wpk[g][:, _o["tspk"]:_o["tspk"] + ADIM] for g in range(2)]
            fspk_s = [wpk[g][:, _o["fspk"]:_o["fspk"] + ODIM] for g in range(2)]
            spk_s = [wpk[g][:, _o["spk"]:_o["spk"] + 1] for g in range(2)]
            fw1_s = wpk[0][:ODIM, _o["fw1"]:_o["fw1"] + 3 * ADIM]
            tb1_s = [bpk[g][:, 0:1] for g in range(2)]
            tb2_s = [bpk[g][:, 1:2] for g in range(2)]
            fb1_s = [bpk[g][:, 2:3] for g in range(2)]
            fb2_s = [bpk[g][:, 3:4] for g in range(2)]
            fb3_s = [bpk[g][:, 4:5] for g in range(2)]

            ones_row = wp.tile([1, 128], BF16, tag="ones_row")
            nc.vector.memset(ones_row[:], 1.0)
            ones_col = wp.tile([128, 1], BF16, tag="ones_col")
            nc.vector.memset(ones_col[:], 1.0)

            # ---- input transposes (DMA transpose, bf16) ----
            textsT = []
            for g in range(2):
                t = ap_.tile([128, T_TEXT], BF16, tag=f"textsT_{g}")
                src = texts.rearrange("t (g c) -> t g c", c=128)[:, g, :]
                nc.sync.dma_start(t[:], src, transpose=True)
                textsT.append(t)
            featsT_full = ap_.tile([128, T_FEATS], BF16, tag="featsT")
            featsT = featsT_full[:ODIM, :]

            def emit_featsT(w):  # transpose one 512-row window of feats
                nc.sync.dma_start(featsT_full[:, w * NW:(w + 1) * NW],
                                  feats[w * NW:(w + 1) * NW, :], transpose=True)

            # ---- speaker projections ----
            spk_t_col = []
            for co in range(2):
                ps = convps.tile([128, 1], F32, tag="convps")
                for g in range(2):
                    nc.tensor.matmul(ps[:], tspk_s[g][:, co * 128:(co + 1) * 128],
                                     spk_s[g][:], start=(g == 0), stop=(g == 1))
                c = colp.tile([128, 1], F32, tag=f"spkt_{co}")
                nc.scalar.activation(c[:], ps[:], AF.Copy)
                spk_t_col.append(c)
            psf = convps.tile([ODIM, 1], F32, tag="convps")
            for g in range(2):
                nc.tensor.matmul(psf[:], fspk_s[g][:, :], spk_s[g][:],
                                 start=(g == 0), stop=(g == 1))
            spk_f_col = colp.tile([ODIM, 1], F32, tag="spkf")
            nc.scalar.activation(spk_f_col[:], psf[:], AF.Copy)

            # ---- conv inputs with speaker bias, zero-padded for K=3 ----
            texts_c = []
            for g in range(2):
                t = ap_.tile([128, T_TEXT + 2], BF16, tag=f"texts_c_{g}")
                nc.vector.memset(t[:, 0:1], 0.0)
                nc.vector.memset(t[:, T_TEXT + 1:T_TEXT + 2], 0.0)
                nc.scalar.activation(t[:, 1:T_TEXT + 1], textsT[g][:], AF.Identity,
                                     bias=spk_t_col[g][:])
                texts_c.append(t)
            feats_c = ap_.tile([ODIM, T_FEATS + 2], BF16, tag="feats_c")
            nc.vector.memset(feats_c[:, 0:1], 0.0)
            nc.vector.memset(feats_c[:, T_FEATS + 1:T_FEATS + 2], 0.0)

            def emit_feats_c(w):  # speaker-bias one window into the padded tile
                nc.scalar.activation(feats_c[:, 1 + w * NW:1 + (w + 1) * NW],
                                     featsT[:, w * NW:(w + 1) * NW], AF.Identity,
                                     bias=spk_f_col[:])

            # ---- text conv1 (K=3) + relu ----
            h_text = []
            for co in range(2):
                t = ap_.tile([128, T_TEXT + 2], BF16, tag=f"h_text_{co}")
                nc.vector.memset(t[:, 0:1], 0.0)
                nc.vector.memset(t[:, T_TEXT + 1:T_TEXT + 2], 0.0)
                h_text.append(t)
            for co in range(2):
                pss = [convps.tile([128, NW], F32, tag="convps", name="tc1ps")
                       for _ in range(2)]
                for wi, (g, k) in enumerate([(g, k) for g in range(2) for k in range(3)]):
                    for n in range(2):
                        nc.tensor.matmul(
                            pss[n][:], tw1_s[g][:, k * ADIM + co * 128: k * ADIM + co * 128 + 128],
                            texts_c[g][:, n * NW + k: n * NW + k + NW],
                            start=(wi == 0), stop=(wi == 5))
                for n in range(2):
                    nc.vector.tensor_scalar(h_text[co][:, 1 + n * NW: 1 + n * NW + NW],
                                            pss[n][:], tb1_s[co][:], 0.0,
                                            AluOpType.add, AluOpType.max)

            # ---- text conv2 (K=1) -> te ; sq = te*te ----
            te = [ap_.tile([128, T_TEXT], BF16, tag=f"te_{co}", name=f"te_{co}") for co in range(2)]
            sq = [ap_.tile([128, T_TEXT], BF16, tag=f"sq_{co}", name=f"sq_{co}") for co in range(2)]
            for co in range(2):
                pss = [convps.tile([128, NW], F32, tag="convps", name="tc2ps")
                       for _ in range(2)]
                for g in range(2):
                    for n in range(2):
                        nc.tensor.matmul(pss[n][:], tw2_s[g][:, co * 128:co * 128 + 128],
                                         h_text[g][:, 1 + n * NW: 1 + n * NW + NW],
                                         start=(g == 0), stop=(g == 1))
                for n in range(2):
                    nc.scalar.activation(te[co][:, n * NW:(n + 1) * NW], pss[n][:],
                                         AF.Identity, bias=tb2_s[co][:])
                nc.vector.tensor_tensor(sq[co][:], te[co][:], te[co][:], AluOpType.mult)

            # ---- negTt2 row: -T * sum_c te^2 ----
            negTt2 = ap_.tile([1, T_TEXT], BF16, tag="negTt2")
            for n in range(T_TEXT // NW):
                ps = convps.tile([1, NW], F32, tag="convps")
                for g in range(2):
                    nc.tensor.matmul(ps[:], ones_col[:], sq[g][:, n * NW:(n + 1) * NW],
                                     start=(g == 0), stop=(g == 1))
                nc.scalar.activation(negTt2[:, n * NW:(n + 1) * NW], ps[:], AF.Copy,
                                     scale=-TEMPERATURE)

            # ---- persistent feat activations (padded for K=3 halos) ----
            h1, h2 = [], []
            for co in range(2):
                t = ap_.tile([128, T_FEATS + 2], BF16, tag=f"h1_{co}", name=f"h1_{co}")
                nc.vector.memset(t[:, 0:1], 0.0)
                nc.vector.memset(t[:, T_FEATS + 1:T_FEATS + 2], 0.0)
                h1.append(t)
                t = ap_.tile([128, T_FEATS + 2], BF16, tag=f"h2_{co}", name=f"h2_{co}")
                nc.vector.memset(t[:, 0:1], 0.0)
                nc.vector.memset(t[:, T_FEATS + 1:T_FEATS + 2], 0.0)
                h2.append(t)
            fe = [ap_.tile([128, T_FEATS], BF16, tag=f"fe_{co}", name=f"fe_{co}") for co in range(2)]

            def emit_conv1(w):  # feats_c -> h1, window w
                for co in range(2):
                    ps = convps.tile([128, NW], F32, tag="convps", name="fc1ps")
                    for k in range(3):
                        nc.tensor.matmul(
                            ps[:], fw1_s[:, k * ADIM + co * 128: k * ADIM + co * 128 + 128],
                            feats_c[:, w * NW + k: w * NW + k + NW],
                            start=(k == 0), stop=(k == 2))
                    nc.vector.tensor_scalar(h1[co][:, 1 + w * NW: 1 + w * NW + NW],
                                            ps[:], fb1_s[co][:], 0.0,
                                            AluOpType.add, AluOpType.max)

            def emit_conv2(w):  # h1 -> h2, window w
                for co in range(2):
                    ps = convps.tile([128, NW], F32, tag="convps", name="fc2ps")
                    first = True
                    for g in range(2):
                        for k in range(3):
                            nc.tensor.matmul(
                                ps[:], fw2_s[g][:, k * ADIM + co * 128: k * ADIM + co * 128 + 128],
                                h1[g][:, w * NW + k: w * NW + k + NW],
                                start=first, stop=(g == 1 and k == 2))
                            first = False
                    nc.scalar.activation(h2[co][:, 1 + w * NW: 1 + w * NW + NW], ps[:],
                                         AF.Relu, bias=fb2_s[co][:])

            def emit_conv3(w):  # h2 -> fe (pre-scaled by 2T), window w
                for co in range(2):
                    ps = convps.tile([128, NW], F32, tag="convps", name="fc3ps")
                    for g in range(2):
                        nc.tensor.matmul(ps[:], fw3_s[g][:, co * 128:co * 128 + 128],
                                         h2[g][:, 1 + w * NW: 1 + w * NW + NW],
                                         start=(g == 0), stop=(g == 1))
                    nc.scalar.activation(fe[co][:, w * NW:(w + 1) * NW], ps[:],
                                         AF.Identity, bias=fb3_s[co][:])

            def emit_chunk(c):  # one 128-feat-row attention chunk
                rows = slice(c * 128, (c + 1) * 128)
                s_ps = spsum.tile([128, T_TEXT], F32, tag="s", name="s_ps")
                sls = [slice(n * NW, (n + 1) * NW) for n in range(T_TEXT // NW)]
                for sl in sls:
                    nc.tensor.matmul(s_ps[:, sl], ones_row[:], negTt2[:, sl],
                                     start=True, stop=False)
                for g in range(2):
                    for sl in sls:
                        nc.tensor.matmul(s_ps[:, sl], fe[g][:, rows], te[g][:, sl],
                                         start=False, stop=(g == 1))

                e1 = cp.tile([128, T_TEXT], BF16, tag="e1", name="e1")
                z1 = colp.tile([128, 1], F32, tag="z1", name="z1")
                nc.scalar.activation(e1[:], s_ps[:], AF.Exp, accum_out=z1[:])
                logz1 = colp.tile([128, 1], F32, tag="logz1", name="logz1")
                nc.scalar.activation(logz1[:], z1[:], AF.Ln)

                lp = cp.tile([128, T_TEXT], F16, tag="lp", name="lp")
                nc.sync.dma_start(lp[:], lp_in[rows, :])
                pm = cp.tile([128, T_TEXT], F16, tag="pm", name="pm")
                nc.sync.dma_start(pm[:], pm_in[rows, :])

                alp = cp.tile([128, T_TEXT], F16, tag="alp", name="alp")
                nc.vector.scalar_tensor_tensor(alp[:], s_ps[:], logz1[:], lp[:],
                                               AluOpType.subtract, AluOpType.add)
                nc.sync.dma_start(alp_out[rows, :], alp[:])

                # attn = (e1*pm) / sum(e1*pm): the 1/Z1 factor cancels, so the
                # second softmax feeds straight off e1 - no second Exp pass.
                attn_u = cp.tile([128, T_TEXT], BF16, tag="attn_u", name="attn_u")
                z2 = colp.tile([128, 1], F32, tag="z2", name="z2")
                nc.vector.scalar_tensor_tensor(attn_u[:], e1[:], 0.0, pm[:],
                                               AluOpType.add, AluOpType.mult,
                                               accum_out=z2[:])
                rz2 = colp.tile([128, 1], F32, tag="rz2", name="rz2")
                nc.vector.reciprocal(rz2[:], z2[:])
                attn = cp.tile([128, T_TEXT], F16, tag="attn", name="attn")
                nc.vector.tensor_scalar_mul(attn[:], attn_u[:], rz2[:])
                nc.sync.dma_start(attn_out[rows, :], attn[:])

            # ---- software-pipelined: transpose(w+1) -> conv1(w) -> conv2(w-1)
            #      -> conv3(w-2) -> 4 attention chunks of window w-2 ----
            emit_featsT(0)
            emit_feats_c(0)
            for w in range(NWIN + 2):
                if w + 1 < NWIN:
                    emit_featsT(w + 1)
                    emit_feats_c(w + 1)
                if w < NWIN:
                    emit_conv1(w)
                if 1 <= w <= NWIN:
                    emit_conv2(w - 1)
                if w >= 2:
                    emit_conv3(w - 2)
                    for i in range(4):
                        emit_chunk(4 * (w - 2) + i)

    nc.finalize()
    return nc


def prep_inputs(inputs):
    bf = ml_dtypes.bfloat16
    T2 = 2.0 * TEMPERATURE

    def to_lhsT(w):  # (O, I, K) -> (I, K*O)
        O, I, K = w.shape
        return np.ascontiguousarray(w.transpose(1, 2, 0).reshape(I, K * O))

    wpack = np.zeros((ADIM, WPACK_W), np.float32)
    o = WPACK_OFFS
    wpack[:, o["tw1"]:o["tw1"] + 3 * ADIM] = to_lhsT(inputs["text_w1"])
    wpack[:, o["tw2"]:o["tw2"] + ADIM] = inputs["text_w2"][:, :, 0].T
    wpack[:, o["fw2"]:o["fw2"] + 3 * ADIM] = to_lhsT(inputs["feat_w2"])
    wpack[:, o["fw3"]:o["fw3"] + ADIM] = inputs["feat_w3"][:, :, 0].T * T2
    wpack[:, o["tspk"]:o["tspk"] + ADIM] = inputs["text_spk_w"].T
    wpack[:, o["fspk"]:o["fspk"] + ODIM] = inputs["feat_spk_w"].T
    wpack[:ODIM, o["fw1"]:o["fw1"] + 3 * ADIM] = to_lhsT(inputs["feat_w1"])
    bpack = np.stack([inputs["text_b1"], inputs["text_b2"], inputs["feat_b1"],
                      inputs["feat_b2"], inputs["feat_b3"] * T2], axis=1).astype(np.float32)
    shared = {"bpack": bpack}
    in_maps = []
    for b in range(NCORES):
        m = dict(shared)
        wp_b = wpack.copy()
        wp_b[:, o["spk"]] = inputs["speaker_embed"][b]
        m["wpack"] = wp_b.astype(bf)
        m["texts"] = np.ascontiguousarray(inputs["texts"][b]).astype(bf)
        fpad = np.zeros((T_FEATS, 128), np.float32)
        fpad[:, :ODIM] = inputs["feats"][b]
        m["feats"] = fpad.astype(bf)
        pr = inputs["attn_prior"][b].astype(np.float64) + EPS
        m["lp"] = np.log(pr).astype(np.float16)
        valid = (~inputs["x_masks"][b, :, 0]).astype(np.float64)
        m["pm"] = (pr * valid[None, :]).astype(np.float16)
        in_maps.append(m)
    return in_maps


def run(inputs, **kwargs):
    nc = build_program()
    in_maps = prep_inputs({k: np.asarray(v) for k, v in inputs.items()})
    res = bass_utils.run_bass_kernel_spmd(nc, in_maps, core_ids=list(range(NCORES)),
                                          **kwargs)
    outs = res.results
    attn = np.stack([outs[b]["attn"] for b in range(NCORES)])[:, None]
    alp = np.stack([outs[b]["alp"] for b in range(NCORES)])[:, None]
    return (attn.astype(np.float32), alp.astype(np.float32)), res


def kernel(**inputs):
    (attn, alp), _ = run(inputs)
    return attn, alp


# revision 19
# speedup vs baseline: 1.0875x; 1.0875x over previous
"""AlignmentModule kernel for 8 TRN2 NeuronCores.

Sharding: data-parallel over batch B=8 -> one batch element per core.

Per-core math (validated numerically against the reference):
  spk_t = text_spk_w @ e ; spk_f = feat_spk_w @ e
  texts_c = texts.T + spk_t[:,None] ; feats_c = feats.T + spk_f[:,None]
  te = conv1x(relu(conv3(texts_c)))                       (256, 1024)
  fe = 2T * conv1x(relu(conv3(relu(conv3(feats_c)))))     (256, 4096)  [2T folded into w3/b3]
  t2[t] = sum_c te[c,t]^2
  s[f,t] = fe.T @ te - T*t2[t]          == -T*dist + T*f2  (f2 cancels in both softmaxes)
  logZ1 = ln(sum_t exp(s))              (s in [-0.25, 0] -> no max-shift needed)
  alp = s - logZ1 + lp                  -> output attn_logprob   [lp = ln(prior+eps), host-prepped]
  attn = (exp(alp) * mask01) / sum_t(exp(alp) * mask01)   -> output attn

The feat conv chain is software-pipelined with the attention chunks over
8 t-windows of 512 so PE/ACT/DVE/DMA overlap and the PE HAM clock-gate
stays warm (2.4 GHz).
"""

import sys

import numpy as np

if "/opt/trn_rl_repo" not in sys.path:
    sys.path.append("/opt/trn_rl_repo")

import ml_dtypes

import concourse.bass as bass
import concourse.bacc as bacc
import concourse.mybir as mybir
import concourse.tile as tile
from concourse import bass_utils
from concourse.alu_op_type import AluOpType

F32 = mybir.dt.float32
F16 = mybir.dt.float16
BF16 = mybir.dt.bfloat16
AF = mybir.ActivationFunctionType

B, T_TEXT, T_FEATS, ADIM, ODIM = 8, 1024, 4096, 256, 80
TEMPERATURE = 0.0005
EPS = 1e-8
NCORES = 8
FCH = T_FEATS // 128  # 32 attention row-chunks per core
NW = 512              # matmul moving width
NWIN = T_FEATS // NW  # 8 feat windows, 4 chunks each

# packed-weight column layout (all lhsT, cin rows x concatenated cols)
WPACK_OFFS = {}
_off = 0
for _nm, _w in [("tw1", 3 * ADIM), ("tw2", ADIM), ("fw2", 3 * ADIM), ("fw3", ADIM),
                ("tspk", ADIM), ("fspk", ODIM), ("spk", 1), ("fw1", 3 * ADIM)]:
    WPACK_OFFS[_nm] = _off
    _off += _w
WPACK_W = _off


def _patched_tables(arch):
    """Steer bacc's act-table pass to the one set containing every function
    we use (Exp, Ln, Identity, Relu, Copy) so the kernel does a single
    ACT_TABLE_LOAD instead of ping-ponging sets every chunk (~2.7us each).
    Set ids stay positional (full dict, original order) - only membership of
    the other sets is masked so the chooser can't pick them."""
    t = _orig_tables(arch)
    need = {AF.Exp, AF.Ln, AF.Identity, AF.Relu, AF.Copy}
    return {name: (set(fns) if name == "natural_log_exp_and_others"
                   else set(fns) - need)
            for name, fns in t.items()}


_orig_tables = bacc.get_activation_tables


def build_program():
    bacc.get_activation_tables = _patched_tables
    try:
        return _build_program_inner()
    finally:
        bacc.get_activation_tables = _orig_tables


def _build_program_inner():
    nc = bacc.Bacc("TRN2", target_bir_lowering=False, debug=False)

    # ---- DRAM I/O ----
    texts = nc.dram_tensor("texts", [T_TEXT, ADIM], BF16, kind="ExternalInput").ap()
    # feats padded 80 -> 128 channels on host (DMA transpose needs free%128==0)
    feats = nc.dram_tensor("feats", [T_FEATS, 128], BF16, kind="ExternalInput").ap()
    lp_in = nc.dram_tensor("lp", [T_FEATS, T_TEXT], F16, kind="ExternalInput").ap()
    pm_in = nc.dram_tensor("pm", [T_FEATS, T_TEXT], F16, kind="ExternalInput").ap()
    wpack = nc.dram_tensor("wpack", [ADIM, WPACK_W], BF16, kind="ExternalInput").ap()
    bpack = nc.dram_tensor("bpack", [ADIM, 5], F32, kind="ExternalInput").ap()

    attn_out = nc.dram_tensor("attn", [T_FEATS, T_TEXT], F16, kind="ExternalOutput").ap()
    alp_out = nc.dram_tensor("alp", [T_FEATS, T_TEXT], F16, kind="ExternalOutput").ap()

    with tile.TileContext(nc) as tc:
        with (
            tc.tile_pool(name="wpool", bufs=1) as wp,
            tc.tile_pool(name="actpool", bufs=1) as ap_,
            tc.tile_pool(name="chunk", bufs=6) as cp,
            tc.tile_pool(name="cols", bufs=8) as colp,
            tc.tile_pool(name="convps", bufs=2, space="PSUM") as convps,
            tc.tile_pool(name="spsum", bufs=3, space="PSUM") as spsum,
        ):
            # ---- load weights / constants into SBUF (packed: 4 DMAs) ----
            wpk = [wp.tile([128, WPACK_W], BF16, tag=f"wpk_{g}", name=f"wpk_{g}")
                   for g in range(2)]
            bpk = [wp.tile([128, 5], F32, tag=f"bpk_{g}", name=f"bpk_{g}")
                   for g in range(2)]
            for g in range(2):
                nc.sync.dma_start(wpk[g][:], wpack[g * 128:(g + 1) * 128, :])
                nc.sync.dma_start(bpk[g][:], bpack[g * 128:(g + 1) * 128, :])

            def wcut(width):
                off = [0]
                def cut(g):
                    o = off[0]
                    return None  # placeholder
                return None

            _o = WPACK_OFFS
            tw1_s = [wpk[g][:, _o["tw1"]:_o["tw1"] + 3 * ADIM] for g in range(2)]
            tw2_s = [wpk[g][:, _o["tw2"]:_o["tw2"] + ADIM] for g in range(2)]
            fw2_s = [wpk[g][:, _o["fw2"]:_o["fw2"] + 3 * ADIM] for g in range(2)]
            fw3_s = [wpk[g][:, _o["fw3"]:_o["fw3"] + ADIM] for g in range(2)]
            tspk_s = [wpk[g][:, _o["tspk"]:_o["tspk"] + ADIM] for g in range(2)]
            fspk_s = [wpk[g][:, _o["fspk"]:_o["fspk"] + ODIM] for g in range(2)]
            spk_s = [wpk[g][:, _o["spk"]:_o["spk"] + 1] for g in range(2)]
            fw1_s = wpk[0][:ODIM, _o["fw1"]:_o["fw1"] + 3 * ADIM]
            tb1_s = [bpk[g][:, 0:1] for g in range(2)]
            tb2_s = [bpk[g][:, 1:2] for g in range(2)]
            fb1_s = [bpk[g][:, 2:3] for g in range(2)]
            fb2_s = [bpk[g][:, 3:4] for g in range(2)]
            fb3_s = [bpk[g][:, 4:5] for g in range(2)]

            ones_row = wp.tile([1, 128], BF16, tag="ones_row")
            nc.vector.memset(ones_row[:], 1.0)
            ones_col = wp.tile([128, 1], BF16, tag="ones_col")
            nc.vector.memset(ones_col[:], 1.0)

            # ---- input transposes (DMA transpose, bf16) ----
            textsT = []
            for g in range(2):
                t = ap_.tile([128, T_TEXT], BF16, tag=f"textsT_{g}")
                src = texts.rearrange("t (g c) -> t g c", c=128)[:, g, :]
                nc.sync.dma_start(t[:], src, transpose=True)
                textsT.append(t)
            featsT_full = ap_.tile([128, T_FEATS], BF16, tag="featsT")
            featsT = featsT_full[:ODIM, :]

            def emit_featsT(w):  # transpose one 512-row window of feats
                nc.sync.dma_start(featsT_full[:, w * NW:(w + 1) * NW],
                                  feats[w * NW:(w + 1) * NW, :], transpose=True)

            # ---- speaker projections ----
            spk_t_col = []
            for co in range(2):
                ps = convps.tile([128, 1], F32, tag="convps")
                for g in range(2):
                    nc.tensor.matmul(ps[:], tspk_s[g][:, co * 128:(co + 1) * 128],
                                     spk_s[g][:], start=(g == 0), stop=(g == 1))
                c = colp.tile([128, 1], F32, tag=f"spkt_{co}")
                nc.scalar.activation(c[:], ps[:], AF.Copy)
                spk_t_col.append(c)
            psf = convps.tile([ODIM, 1], F32, tag="convps")
            for g in range(2):
                nc.tensor.matmul(psf[:], fspk_s[g][:, :], spk_s[g][:],
                                 start=(g == 0), stop=(g == 1))
            spk_f_col = colp.tile([ODIM, 1], F32, tag="spkf")
            nc.scalar.activation(spk_f_col[:], psf[:], AF.Copy)

            # ---- conv inputs with speaker bias, zero-padded for K=3 ----
            texts_c = []
            for g in range(2):
                t = ap_.tile([128, T_TEXT + 2], BF16, tag=f"texts_c_{g}")
                nc.vector.memset(t[:, 0:1], 0.0)
                nc.vector.memset(t[:, T_TEXT + 1:T_TEXT + 2], 0.0)
                nc.scalar.activation(t[:, 1:T_TEXT + 1], textsT[g][:], AF.Identity,
                                     bias=spk_t_col[g][:])
                texts_c.append(t)
            feats_c = ap_.tile([ODIM, T_FEATS + 2], BF16, tag="feats_c")
            nc.vector.memset(feats_c[:, 0:1], 0.0)
            nc.vector.memset(feats_c[:, T_FEATS + 1:T_FEATS + 2], 0.0)

            def emit_feats_c(w):  # speaker-bias one window into the padded tile
                nc.scalar.activation(feats_c[:, 1 + w * NW:1 + (w + 1) * NW],
                                     featsT[:, w * NW:(w + 1) * NW], AF.Identity,
                                     bias=spk_f_col[:])

            # ---- text conv1 (K=3) + relu ----
            h_text = []
            for co in range(2):
                t = ap_.tile([128, T_TEXT + 2], BF16, tag=f"h_text_{co}")
                nc.vector.memset(t[:, 0:1], 0.0)
                nc.vector.memset(t[:, T_TEXT + 1:T_TEXT + 2], 0.0)
                h_text.append(t)
            for co in range(2):
                pss = [convps.tile([128, NW], F32, tag="convps", name="tc1ps")
                       for _ in range(2)]
                for wi, (g, k) in enumerate([(g, k) for g in range(2) for k in range(3)]):
                    for n in range(2):
                        nc.tensor.matmul(
                            pss[n][:], tw1_s[g][:, k * ADIM + co * 128: k * ADIM + co * 128 + 128],
                            texts_c[g][:, n * NW + k: n * NW + k + NW],
                            start=(wi == 0), stop=(wi == 5))
                for n in range(2):
                    nc.vector.tensor_scalar(h_text[co][:, 1 + n * NW: 1 + n * NW + NW],
                                            pss[n][:], tb1_s[co][:], 0.0,
                                            AluOpType.add, AluOpType.max)

            # ---- text conv2 (K=1) -> te ; sq = te*te ----
            te = [ap_.tile([128, T_TEXT], BF16, tag=f"te_{co}", name=f"te_{co}") for co in range(2)]
            sq = [ap_.tile([128, T_TEXT], BF16, tag=f"sq_{co}", name=f"sq_{co}") for co in range(2)]
            for co in range(2):
                pss = [convps.tile([128, NW], F32, tag="convps", name="tc2ps")
                       for _ in range(2)]
                for g in range(2):
                    for n in range(2):
                        nc.tensor.matmul(pss[n][:], tw2_s[g][:, co * 128:co * 128 + 128],
                                         h_text[g][:, 1 + n * NW: 1 + n * NW + NW],
                                         start=(g == 0), stop=(g == 1))
                for n in range(2):
                    nc.scalar.activation(te[co][:, n * NW:(n + 1) * NW], pss[n][:],
                                         AF.Identity, bias=tb2_s[co][:])
                nc.vector.tensor_tensor(sq[co][:], te[co][:], te[co][:], AluOpType.mult)

            # ---- negTt2 row: -T * sum_c te^2 ----
            negTt2 = ap_.tile([1, T_TEXT], BF16, tag="negTt2")
            for n in range(T_TEXT // NW):
                ps = convps.tile([1, NW], F32, tag="convps")
                for g in range(2):
                    nc.tensor.matmul(ps[:], ones_col[:], sq[g][:, n * NW:(n + 1) * NW],
                                     start=(g == 0), stop=(g == 1))
                nc.scalar.activation(negTt2[:, n * NW:(n + 1) * NW], ps[:], AF.Copy,
                                     scale=-TEMPERATURE)

            # ---- persistent feat activations (padded for K=3 halos) ----
            h1, h2 = [], []
            for co in range(2):
                t = ap_.tile([128, T_FEATS + 2], BF16, tag=f"h1_{co}", name=f"h1_{co}")
                nc.vector.memset(t[:, 0:1], 0.0)
                nc.vector.memset(t[:, T_FEATS + 1:T_FEATS + 2], 0.0)
                h1.append(t)
                t = ap_.tile([128, T_FEATS + 2], BF16, tag=f"h2_{co}", name=f"h2_{co}")
                nc.vector.memset(t[:, 0:1], 0.0)
                nc.vector.memset(t[:, T_FEATS + 1:T_FEATS + 2], 0.0)
                h2.append(t)
            fe = [ap_.tile([128, T_FEATS], BF16, tag=f"fe_{co}", name=f"fe_{co}") for co in range(2)]

            def emit_conv1(w):  # feats_c -> h1, window w
                for co in range(2):
                    ps = convps.tile([128, NW], F32, tag="convps", name="fc1ps")
                    for k in range(3):
                        nc.tensor.matmul(
                            ps[:], fw1_s[:, k * ADIM + co * 128: k * ADIM + co * 128 + 128],
                            feats_c[:, w * NW + k: w * NW + k + NW],
                            start=(k == 0), stop=(k == 2))
                    nc.scalar.activation(h1[co][:, 1 + w * NW: 1 + w * NW + NW],
                                         ps[:], AF.Relu, bias=fb1_s[co][:])

            def emit_conv2(w):  # h1 -> h2, window w
                for co in range(2):
                    ps = convps.tile([128, NW], F32, tag="convps", name="fc2ps")
                    first = True
                    for g in range(2):
                        for k in range(3):
                            nc.tensor.matmul(
                                ps[:], fw2_s[g][:, k * ADIM + co * 128: k * ADIM + co * 128 + 128],
                                h1[g][:, w * NW + k: w * NW + k + NW],
                                start=first, stop=(g == 1 and k == 2))
                            first = False
                    nc.scalar.activation(h2[co][:, 1 + w * NW: 1 + w * NW + NW], ps[:],
                                         AF.Relu, bias=fb2_s[co][:])

            def emit_conv3(w):  # h2 -> fe (pre-scaled by 2T), window w
                for co in range(2):
                    ps = convps.tile([128, NW], F32, tag="convps", name="fc3ps")
                    for g in range(2):
                        nc.tensor.matmul(ps[:], fw3_s[g][:, co * 128:co * 128 + 128],
                                         h2[g][:, 1 + w * NW: 1 + w * NW + NW],
                                         start=(g == 0), stop=(g == 1))
                    nc.scalar.activation(fe[co][:, w * NW:(w + 1) * NW], ps[:],
                                         AF.Identity, bias=fb3_s[co][:])

            def emit_chunk(c):  # one 128-feat-row attention chunk
                rows = slice(c * 128, (c + 1) * 128)
                lp = cp.tile([128, T_TEXT], F16, tag="lp", name="lp")
                nc.sync.dma_start(lp[:], lp_in[rows, :])
                pm = cp.tile([128, T_TEXT], F16, tag="pm", name="pm")
                nc.sync.dma_start(pm[:], pm_in[rows, :])
                s_ps = spsum.tile([128, T_TEXT], F32, tag="s", name="s_ps")
                sls = [slice(n * NW, (n + 1) * NW) for n in range(T_TEXT // NW)]
                for sl in sls:
                    nc.tensor.matmul(s_ps[:, sl], ones_row[:], negTt2[:, sl],
                                     start=True, stop=False)
                for g in range(2):
                    for sl in sls:
                        nc.tensor.matmul(s_ps[:, sl], fe[g][:, rows], te[g][:, sl],
                                         start=False, stop=(g == 1))

                e1 = cp.tile([128, T_TEXT], BF16, tag="e1", name="e1")
                z1 = colp.tile([128, 1], F32, tag="z1", name="z1")
                nc.scalar.activation(e1[:], s_ps[:], AF.Exp, accum_out=z1[:])
                logz1 = colp.tile([128, 1], F32, tag="logz1", name="logz1")
                nc.scalar.activation(logz1[:], z1[:], AF.Ln)


                alp = cp.tile([128, T_TEXT], F16, tag="alp", name="alp")
                nc.vector.scalar_tensor_tensor(alp[:], s_ps[:], logz1[:], lp[:],
                                               AluOpType.subtract, AluOpType.add)
                nc.sync.dma_start(alp_out[rows, :], alp[:])

                # attn = (e1*pm) / sum(e1*pm): the 1/Z1 factor cancels, so the
                # second softmax feeds straight off e1 - no second Exp pass.
                attn_u = cp.tile([128, T_TEXT], BF16, tag="attn_u", name="attn_u")
                z2 = colp.tile([128, 1], F32, tag="z2", name="z2")
                nc.vector.scalar_tensor_tensor(attn_u[:], e1[:], 0.0, pm[:],
                                               AluOpType.add, AluOpType.mult,
                                               accum_out=z2[:])
                rz2 = colp.tile([128, 1], F32, tag="rz2", name="rz2")
                nc.vector.reciprocal(rz2[:], z2[:])
                attn = cp.tile([128, T_TEXT], F16, tag="attn", name="attn")
                nc.scalar.activation(attn[:], attn_u[:], AF.Copy, scale=rz2[:])
                nc.sync.dma_start(attn_out[rows, :], attn[:])

            # ---- software-pipelined: transpose(w+1) -> conv1(w) -> conv2(w-1)
            #      -> conv3(w-2) -> 4 attention chunks of window w-2 ----
            emit_featsT(0)
            emit_feats_c(0)
            for w in range(NWIN + 2):
                if w + 1 < NWIN:
                    emit_featsT(w + 1)
                    emit_feats_c(w + 1)
                if w < NWIN:
                    emit_conv1(w)
                if 1 <= w <= NWIN:
                    emit_conv2(w - 1)
                if w >= 2:
                    emit_conv3(w - 2)
                    for i in range(4):
                        emit_chunk(4 * (w - 2) + i)

    nc.finalize()
    return nc


def prep_inputs(inputs):
    bf = ml_dtypes.bfloat16
    T2 = 2.0 * TEMPERATURE

    def to_lhsT(w):  # (O, I, K) -> (I, K*O)
        O, I, K = w.shape
        return np.ascontiguousarray(w.transpose(1, 2, 0).reshape(I, K * O))

    wpack = np.zeros((ADIM, WPACK_W), np.float32)
    o = WPACK_OFFS
    wpack[:, o["tw1"]:o["tw1"] + 3 * ADIM] = to_lhsT(inputs["text_w1"])
    wpack[:, o["tw2"]:o["tw2"] + ADIM] = inputs["text_w2"][:, :, 0].T
    wpack[:, o["fw2"]:o["fw2"] + 3 * ADIM] = to_lhsT(inputs["feat_w2"])
    wpack[:, o["fw3"]:o["fw3"] + ADIM] = inputs["feat_w3"][:, :, 0].T * T2
    wpack[:, o["tspk"]:o["tspk"] + ADIM] = inputs["text_spk_w"].T
    wpack[:, o["fspk"]:o["fspk"] + ODIM] = inputs["feat_spk_w"].T
    wpack[:ODIM, o["fw1"]:o["fw1"] + 3 * ADIM] = to_lhsT(inputs["feat_w1"])
    bpack = np.stack([inputs["text_b1"], inputs["text_b2"], inputs["feat_b1"],
                      inputs["feat_b2"], inputs["feat_b3"] * T2], axis=1).astype(np.float32)
    shared = {"bpack": bpack}
    in_maps = []
    for b in range(NCORES):
        m = dict(shared)
        wp_b = wpack.copy()
        wp_b[:, o["spk"]] = inputs["speaker_embed"][b]
        m["wpack"] = wp_b.astype(bf)
        m["texts"] = np.ascontiguousarray(inputs["texts"][b]).astype(bf)
        fpad = np.zeros((T_FEATS, 128), np.float32)
        fpad[:, :ODIM] = inputs["feats"][b]
        m["feats"] = fpad.astype(bf)
        pr = inputs["attn_prior"][b].astype(np.float64) + EPS
        m["lp"] = np.log(pr).astype(np.float16)
        valid = (~inputs["x_masks"][b, :, 0]).astype(np.float64)
        m["pm"] = (pr * valid[None, :]).astype(np.float16)
        in_maps.append(m)
    return in_maps


def run(inputs, **kwargs):
    nc = build_program()
    in_maps = prep_inputs({k: np.asarray(v) for k, v in inputs.items()})
    res = bass_utils.run_bass_kernel_spmd(nc, in_maps, core_ids=list(range(NCORES)),
                                          **kwargs)
    outs = res.results
    attn = np.stack([outs[b]["attn"] for b in range(NCORES)])[:, None]
    alp = np.stack([outs[b]["alp"] for b in range(NCORES)])[:, None]
    return (attn.astype(np.float32), alp.astype(np.float32)), res


def kernel(**inputs):
    (attn, alp), _ = run(inputs)
    return attn, alp
